# revision 6
# baseline (speedup 1.0000x reference)
"""Trainium2 Bass kernel for the exact-match memorizer lookup (v3).

Dense PE brute force, queries sharded 512/core, memory replicated.

Host prepares (pure layout / trivially-derived constants, all bf16-exact):
  ktrh [16, 32768] bf16: contraction-major augmented key matrix.
      Rows 0-7:  k_d (features, ints 0..3)
      Row  8:    |k|^2 (int <= 72)
      Row  9:    0.25            (ramp base, paired with x-side 1)
      Row 10:    -A * 2^-8       (A = (j mod 2048) >> 5, 6 bits)
      Row 11:    -B * 2^-13      (B = j mod 32, 5 bits)
      Row 12:    1.0             (|x|^2 carrier)
      Rows 13-15: 0
  xaugT [16, 512] f32: matching query-side columns:
      [-2 x_d (8), 1, 1, 1, 1, |x|^2, 0, 0, 0]
  mvpad [32768, 16] f32: mem_values broadcast to 64B rows (gather-friendly).

Per core, per query group g (128 queries) and tile t (2048 mem cols):
  one PSUM tile  ps[m, i] = |x|^2 - 2 x.k + |k|^2 + (2048 - i) * 2^-13
(exact in f32: all terms are multiples of 2^-13, total < 2^10).
A single fused DVE tensor_tensor_reduce (elementwise min of the two
1024-col halves + min-accumulate) yields per query the minimum over the
tile: matches give (2048 - i*) * 2^-13 <= 0.25 with i* the LAST matching
column (ramp strictly decreasing in i); non-matches give > 1.
Decode i* = 2048 - m * 8192, take max of (j_global + 1) * found over the
16 tiles, gather mvpad[jstar], select vs the linear fallback x @ w.T + b.
"""

import sys

if "/opt/trn_rl_repo" not in sys.path:
    sys.path.insert(0, "/opt/trn_rl_repo")

import numpy as np

import bass_rust
from concourse.bass import Bass, IndirectOffsetOnAxis
import concourse.tile as tile
from concourse import bass, mybir

N_QUERIES = 4096
N_MEM = 32768
D_FEAT = 8
N_CORES = 8
NQ = N_QUERIES // N_CORES  # 512 queries per core
QS = NQ // 128  # 4 query groups per core
KAUG = 16  # augmented contraction rows
TW = 2048  # mem cols per PSUM tile (4 banks)
NTILE = N_MEM // TW  # 16
MMN = 512  # moving-operand cols per matmul (PSUM out limited to one bank)
RS = 2.0 ** -13  # ramp scale

F32 = mybir.dt.float32
BF16 = mybir.dt.bfloat16
I32 = mybir.dt.int32
U8 = mybir.dt.uint8


def _patch_tile_drain():
    """This container's walrus accepts only one sync-wait per instruction;
    TileContext's teardown drain waits on every used semaphore at once.
    Split it into one drain per semaphore."""
    if getattr(tile.TileContext, "_drain_patched", False):
        return
    from concourse.tile import ScopedClock

    def _drain_and_barrier(self, tick_clock, wait_clock):
        gc = tick_clock.global_clock
        ticks = eval(repr(gc).replace("VectorClock(", "").rstrip(")"))
        for i, t in enumerate(ticks):
            if t <= 0:
                continue
            part = [t if j == i else 0 for j in range(len(ticks))]
            d = self.nc.sync.drain()
            wait_clock.add_sem_waits(
                d.ins, ScopedClock({None: bass_rust.VectorClock(part)})
            )
        self.nc.all_engine_barrier()
        assert self.sems is not None
        popped = self.nc._tile_sem_poison_stack.pop()
        assert popped is self._sem_poison
        self.nc.clear_and_free_semaphores(list(self.sems.allocated().values()))
        self.nc.all_engine_barrier()

    tile.TileContext._drain_and_barrier = _drain_and_barrier
    tile.TileContext._drain_patched = True


def _fix_multiwaits(bir_bytes: bytes) -> bytes:
    """Hoist extra sync-waits onto standalone EventSemaphore instructions
    inserted immediately before the offender (same engine => identical
    in-order blocking semantics)."""
    import json

    bir = json.loads(bir_bytes)
    for f in bir["functions"]:
        for blk in f["blocks"]:
            insts = blk["instructions"]
            out_insts = []
            changed = False
            for inst in insts:
                si = inst.get("sync_info")
                waits = si.get("on_wait", []) if si else []
                if len(waits) > 1:
                    changed = True
                    for k, wv in enumerate(waits[:-1]):
                        out_insts.append(
                            {
                                "debug": inst.get("debug", 0),
                                "engine": inst["engine"],
                                "ins": [],
                                "name": f"{inst['name']}-sw{k}",
                                "opcode": "EventSemaphore",
                                "outs": [],
                                "sync_info": {"on_update": [], "on_wait": [wv]},
                            }
                        )
                    si["on_wait"] = [waits[-1]]
                out_insts.append(inst)
            if changed:
                blk["instructions"] = out_insts
    return json.dumps(bir).encode()


def build_nc(debug: bool = False) -> Bass:
    _patch_tile_drain()
    nc = Bass()
    AX = mybir.AxisListType
    OP = mybir.AluOpType

    x = nc.dram_tensor("x", [NQ, D_FEAT], F32, kind="ExternalInput")
    ktrh = nc.dram_tensor("ktrh", [KAUG, N_MEM], BF16, kind="ExternalInput")
    xaugT = nc.dram_tensor("xaugT", [KAUG, NQ], F32, kind="ExternalInput")
    mvpad = nc.dram_tensor("mvpad", [N_MEM, 16], F32, kind="ExternalInput")
    w = nc.dram_tensor("w", [1, D_FEAT], F32, kind="ExternalInput")
    b = nc.dram_tensor("b", [1], F32, kind="ExternalInput")
    out = nc.dram_tensor("out", [NQ, 1], F32, kind="ExternalOutput")

    with tile.TileContext(nc) as tc:
        with (
            tc.tile_pool(name="sbuf", bufs=1) as pool,
            tc.tile_pool(name="work", bufs=4) as wpool,
            tc.tile_pool(name="psum", bufs=2, space="PSUM") as ppool,
        ):
            # ---- loads ------------------------------------------------------
            # Keys and query columns staged twice, at partition bases 0 and
            # 32: consecutive matmuls alternate PE row groups so LDWEIGHTS
            # for MM i+1 is pulled ahead while MM i streams (no row-group
            # conflict) and matmuls pipeline back-to-back.
            ktr_t = pool.tile([32 + KAUG, N_MEM], BF16, tag="ktr")
            nc.sync.dma_start(out=ktr_t[0:KAUG, :], in_=ktrh[:])
            nc.sync.dma_start(out=ktr_t[32 : 32 + KAUG, :], in_=ktrh[:])

            xaT_f = pool.tile([KAUG, NQ], F32, tag="xaTf")
            nc.sync.dma_start(out=xaT_f[:], in_=xaugT[:])
            xaT = pool.tile([32 + KAUG, NQ], BF16, tag="xaT")
            nc.vector.tensor_copy(out=xaT[0:KAUG, :], in_=xaT_f[:])
            nc.vector.tensor_copy(out=xaT[32 : 32 + KAUG, :], in_=xaT_f[:])

            # x in layout B (q = g*128 + m): for the linear fallback
            xqb_t = pool.tile([128, QS * D_FEAT], F32, tag="xqb")
            nc.sync.dma_start(
                out=xqb_t[:].rearrange("p (g d) -> p g d", d=D_FEAT),
                in_=x[:].rearrange("(g m) d -> m g d", m=128),
            )
            xqb_v = xqb_t[:].rearrange("p (g d) -> p g d", d=D_FEAT)

            w_t = pool.tile([128, D_FEAT], F32, tag="wt")
            nc.sync.dma_start(out=w_t[:], in_=w[0:1, :].to_broadcast([128, D_FEAT]))
            b_t = pool.tile([128, 1], F32, tag="bt")
            nc.sync.dma_start(out=b_t[:], in_=b[None, :].to_broadcast([128, 1]))

            # ---- linear fallback linq[m, g] = x_q . w + b -------------------
            xw_t = pool.tile([128, QS * D_FEAT], F32, tag="xw")
            nc.vector.tensor_tensor(
                out=xw_t[:].rearrange("p (g d) -> p g d", d=D_FEAT),
                in0=xqb_v,
                in1=w_t[:, None, :].to_broadcast([128, QS, D_FEAT]),
                op=OP.mult,
            )
            linq_t = pool.tile([128, QS], F32, tag="linq")
            nc.vector.reduce_sum(
                out=linq_t[:],
                in_=xw_t[:].rearrange("p (g d) -> p g d", d=D_FEAT),
                axis=AX.X,
            )
            nc.vector.tensor_scalar_add(linq_t[:], linq_t[:], b_t[:, 0:1])

            # ---- main loop: matmul -> fused fold+min ------------------------
            # mins[m, g*16 + t] = min over tile t of group g
            mins_t = pool.tile([128, QS * NTILE], F32, tag="mins")

            val_t = pool.tile([128, QS], F32, tag="val")
            kk_t = pool.tile([128, QS], F32, tag="kk")
            ti_t = pool.tile([128, NTILE], I32, tag="ti")
            nc.gpsimd.iota(ti_t[:], [[1, NTILE]], base=0, channel_multiplier=0)
            tbase_t = pool.tile([128, NTILE], F32, tag="tbase")
            nc.vector.tensor_copy(out=tbase_t[:], in_=ti_t[:])
            nc.vector.tensor_scalar(
                out=tbase_t[:], in0=tbase_t[:], scalar1=float(TW), scalar2=1.0,
                op0=OP.mult, op1=OP.add,
            )  # t*2048 + 1

            mmq = 0  # running matmul index for row-group alternation
            for g in range(QS):
                for t in range(NTILE):
                    ps = ppool.tile([128, TW], F32, tag="ps")
                    for k in range(TW // MMN):
                        pb = 32 * (mmq & 1)
                        mmq += 1
                        nc.tensor.matmul(
                            out=ps[:, k * MMN : (k + 1) * MMN],
                            lhsT=xaT[pb : pb + KAUG, g * 128 : (g + 1) * 128],
                            rhs=ktr_t[
                                pb : pb + KAUG,
                                t * TW + k * MMN : t * TW + (k + 1) * MMN,
                            ],
                            start=True,
                            stop=True,
                        )
                    nc.vector.tensor_reduce(
                        out=mins_t[:, g * NTILE + t : g * NTILE + t + 1],
                        in_=ps[:, None, :],
                        axis=AX.X,
                        op=OP.min,
                    )

                # ---- per-group decode + gather ------------------------------
                gm = mins_t[:, g * NTILE : (g + 1) * NTILE]
                fo = wpool.tile([128, NTILE], F32, tag="fo")
                nc.vector.tensor_scalar(
                    out=fo[:], in0=gm, scalar1=0.5, scalar2=None, op0=OP.is_lt
                )
                ii = wpool.tile([128, NTILE], F32, tag="ii")
                nc.vector.tensor_scalar(
                    out=ii[:], in0=gm, scalar1=-8192.0, scalar2=float(TW),
                    op0=OP.mult, op1=OP.add,
                )  # i* = 2048 - m*8192
                nc.vector.tensor_tensor(out=ii[:], in0=ii[:], in1=tbase_t[:], op=OP.add)
                nc.vector.tensor_tensor(out=ii[:], in0=ii[:], in1=fo[:], op=OP.mult)
                nc.vector.reduce_max(
                    out=kk_t[:, g : g + 1], in_=ii[:], axis=AX.X
                )  # K = jstar + 1, 0 if not found

                rg = pool.tile([128, 1], F32, name=f"rgf{g}", tag=f"rgf{g}")
                nc.vector.tensor_scalar(
                    out=rg[:], in0=kk_t[:, g : g + 1], scalar1=-1.0, scalar2=0.0,
                    op0=OP.add, op1=OP.max,
                )
                rgi = pool.tile([128, 1], I32, name=f"rgi{g}", tag=f"rgi{g}")
                nc.vector.tensor_copy(out=rgi[:], in_=rg[:])
                vg = pool.tile([128, 16], F32, name=f"vg{g}", tag=f"vg{g}")
                nc.gpsimd.indirect_dma_start(
                    out=vg[:],
                    out_offset=None,
                    in_=mvpad[:],
                    in_offset=IndirectOffsetOnAxis(ap=rgi[:, 0:1], axis=0),
                )
                nc.vector.tensor_copy(out=val_t[:, g : g + 1], in_=vg[:, 0:1])

            # ---- select + store --------------------------------------------
            found_t = pool.tile([128, QS], F32, tag="found")
            nc.vector.tensor_scalar(
                out=found_t[:], in0=kk_t[:], scalar1=0.0, scalar2=None, op0=OP.is_gt
            )
            found_u8 = pool.tile([128, QS], U8, tag="found_u8")
            nc.vector.tensor_copy(out=found_u8[:], in_=found_t[:])
            res_t = pool.tile([128, QS], F32, tag="res")
            nc.vector.select(
                out=res_t[:],
                mask=found_u8[:],
                on_true=val_t[:],
                on_false=linq_t[:],
            )
            nc.sync.dma_start(
                out=out[:].rearrange("(g m) one -> m g one", m=128),
                in_=res_t[:, :, None],
            )

            if debug:
                taps = {
                    "d_mins": mins_t,
                    "d_kk": kk_t,
                    "d_linq": linq_t,
                    "d_val": val_t,
                }
                for name, t_ in taps.items():
                    shp = list(t_[:].shape)
                    dt_ = nc.dram_tensor(name, shp, F32, kind="ExternalOutput")
                    nc.sync.dma_start(out=dt_[:], in_=t_[:])

    return nc


_NC_CACHE: dict[str, Bass] = {}


def _get_nc() -> Bass:
    if "nc" not in _NC_CACHE:
        nc = build_nc()
        orig = nc.to_json_bytes
        nc.to_json_bytes = lambda: _fix_multiwaits(orig())
        _NC_CACHE["nc"] = nc
    return _NC_CACHE["nc"]


def _prep_keys(mem_keys: np.ndarray, mem_values: np.ndarray):
    """Host-side layout prep: contraction-major augmented key matrix (bf16,
    all values exactly representable) and 64B-padded value rows."""
    import ml_dtypes

    j = np.arange(N_MEM)
    i_loc = j % TW
    A = (i_loc >> 5).astype(np.float32)
    B = (i_loc & 31).astype(np.float32)
    ktr = np.zeros((KAUG, N_MEM), dtype=np.float32)
    ktr[0:D_FEAT, :] = mem_keys.T
    ktr[8, :] = (mem_keys * mem_keys).sum(axis=1)
    ktr[9, :] = 0.25
    ktr[10, :] = -A * 2.0 ** -8
    ktr[11, :] = -B * 2.0 ** -13
    ktr[12, :] = 1.0
    ktrh = ktr.astype(ml_dtypes.bfloat16)
    mvpad = np.ascontiguousarray(
        np.repeat(mem_values[:, None], 16, axis=1).astype(np.float32)
    )
    return ktrh, mvpad


def _prep_queries(xc: np.ndarray):
    """Host-side query-side augmented columns [16, 512] f32."""
    xa = np.zeros((KAUG, NQ), dtype=np.float32)
    xa[0:D_FEAT, :] = -2.0 * xc.T
    xa[8, :] = 1.0
    xa[9, :] = 1.0
    xa[10, :] = 1.0
    xa[11, :] = 1.0
    xa[12, :] = (xc * xc).sum(axis=1)
    return xa


def kernel(x, mem_keys, mem_values, w, b):
    from concourse.bass_utils import run_bass_kernel_spmd

    x = np.ascontiguousarray(np.asarray(x, dtype=np.float32))
    mem_keys = np.ascontiguousarray(np.asarray(mem_keys, dtype=np.float32))
    mem_values = np.ascontiguousarray(np.asarray(mem_values, dtype=np.float32))
    w = np.ascontiguousarray(np.asarray(w, dtype=np.float32))
    b = np.ascontiguousarray(np.asarray(b, dtype=np.float32))

    nc = _get_nc()
    ktrh, mvpad = _prep_keys(mem_keys, mem_values)
    in_maps = []
    for c in range(N_CORES):
        xc = x[c * NQ : (c + 1) * NQ]
        in_maps.append(
            {
                "x": xc,
                "ktrh": ktrh,
                "xaugT": _prep_queries(xc),
                "mvpad": mvpad,
                "w": w,
                "b": b,
            }
        )
    res = run_bass_kernel_spmd(nc, in_maps, core_ids=list(range(N_CORES)))
    return np.concatenate([r["out"] for r in res.results], axis=0)


if __name__ == "__main__":
    rng = np.random.default_rng(0)
    mk = rng.integers(0, 4, (N_MEM, D_FEAT)).astype(np.float32)
    xx = rng.integers(0, 4, (N_QUERIES, D_FEAT)).astype(np.float32)
    mv = rng.normal(size=N_MEM).astype(np.float32)
    ww = rng.normal(size=(1, D_FEAT)).astype(np.float32)
    bb = rng.normal(size=(1,)).astype(np.float32)
    got = kernel(xx, mk, mv, ww, bb)
    pow4 = (4 ** np.arange(D_FEAT)).astype(np.int64)
    mc = (mk.astype(np.int64) * pow4).sum(1)
    qc = (xx.astype(np.int64) * pow4).sum(1)
    last = {}
    for jj, c in enumerate(mc):
        last[c] = jj
    exp = np.where(
        np.isin(qc, mc),
        mv[[last.get(c, 0) for c in qc]],
        (xx @ ww.T + bb)[:, 0],
    )[:, None]
    err = np.abs(got - exp).max()
    print("max abs err vs numpy model:", err)


# revision 18
# speedup vs baseline: 1.0947x; 1.0947x over previous
"""Trainium2 Bass kernel for the exact-match memorizer lookup (v3).

Dense PE brute force, queries sharded 512/core, memory replicated.

Host prepares (pure layout / trivially-derived constants, all bf16-exact):
  ktrh [16, 32768] bf16: contraction-major augmented key matrix.
      Rows 0-7:  k_d (features, ints 0..3)
      Row  8:    |k|^2 (int <= 72)
      Row  9:    0.25            (ramp base, paired with x-side 1)
      Row 10:    -A * 2^-8       (A = (j mod 2048) >> 5, 6 bits)
      Row 11:    -B * 2^-13      (B = j mod 32, 5 bits)
      Row 12:    1.0             (|x|^2 carrier)
      Rows 13-15: 0
  xaugT [16, 512] f32: matching query-side columns:
      [-2 x_d (8), 1, 1, 1, 1, |x|^2, 0, 0, 0]
  mvpad [32768, 16] f32: mem_values broadcast to 64B rows (gather-friendly).

Per core, per query group g (128 queries) and tile t (2048 mem cols):
  one PSUM tile  ps[m, i] = |x|^2 - 2 x.k + |k|^2 + (2048 - i) * 2^-13
(exact in f32: all terms are multiples of 2^-13, total < 2^10).
A single fused DVE tensor_tensor_reduce (elementwise min of the two
1024-col halves + min-accumulate) yields per query the minimum over the
tile: matches give (2048 - i*) * 2^-13 <= 0.25 with i* the LAST matching
column (ramp strictly decreasing in i); non-matches give > 1.
Decode i* = 2048 - m * 8192, take max of (j_global + 1) * found over the
16 tiles, gather mvpad[jstar], select vs the linear fallback x @ w.T + b.
"""

import sys

if "/opt/trn_rl_repo" not in sys.path:
    sys.path.insert(0, "/opt/trn_rl_repo")

import numpy as np

import bass_rust
from concourse.bass import Bass, IndirectOffsetOnAxis
import concourse.tile as tile
from concourse import bass, mybir

N_QUERIES = 4096
N_MEM = 32768
D_FEAT = 8
N_CORES = 8
NQ = N_QUERIES // N_CORES  # 512 queries per core
QS = NQ // 128  # 4 query groups per core
KAUG = 16  # augmented contraction rows
TW = 2048  # mem cols per PSUM tile (4 banks)
NTILE = N_MEM // TW  # 16
MMN = 512  # moving-operand cols per matmul (PSUM out limited to one bank)
NBLK3 = (N_MEM // MMN + 2) // 3  # 512-col slots per partition-block (3-way)
RS = 2.0 ** -13  # ramp scale

F32 = mybir.dt.float32
BF16 = mybir.dt.bfloat16
I32 = mybir.dt.int32
U8 = mybir.dt.uint8


def _patch_tile_drain():
    """This container's walrus accepts only one sync-wait per instruction;
    TileContext's teardown drain waits on every used semaphore at once.
    Split it into one drain per semaphore."""
    if getattr(tile.TileContext, "_drain_patched", False):
        return
    from concourse.tile import ScopedClock

    def _drain_and_barrier(self, tick_clock, wait_clock):
        gc = tick_clock.global_clock
        ticks = eval(repr(gc).replace("VectorClock(", "").rstrip(")"))
        for i, t in enumerate(ticks):
            if t <= 0:
                continue
            part = [t if j == i else 0 for j in range(len(ticks))]
            d = self.nc.sync.drain()
            wait_clock.add_sem_waits(
                d.ins, ScopedClock({None: bass_rust.VectorClock(part)})
            )
        self.nc.all_engine_barrier()
        assert self.sems is not None
        popped = self.nc._tile_sem_poison_stack.pop()
        assert popped is self._sem_poison
        self.nc.clear_and_free_semaphores(list(self.sems.allocated().values()))
        self.nc.all_engine_barrier()

    tile.TileContext._drain_and_barrier = _drain_and_barrier
    tile.TileContext._drain_patched = True


def _fix_multiwaits(bir_bytes: bytes) -> bytes:
    """Hoist extra sync-waits onto standalone EventSemaphore instructions
    inserted immediately before the offender (same engine => identical
    in-order blocking semantics)."""
    import json

    bir = json.loads(bir_bytes)
    for f in bir["functions"]:
        for blk in f["blocks"]:
            insts = blk["instructions"]
            out_insts = []
            changed = False
            for inst in insts:
                si = inst.get("sync_info")
                waits = si.get("on_wait", []) if si else []
                if len(waits) > 1:
                    changed = True
                    for k, wv in enumerate(waits[:-1]):
                        out_insts.append(
                            {
                                "debug": inst.get("debug", 0),
                                "engine": inst["engine"],
                                "ins": [],
                                "name": f"{inst['name']}-sw{k}",
                                "opcode": "EventSemaphore",
                                "outs": [],
                                "sync_info": {"on_update": [], "on_wait": [wv]},
                            }
                        )
                    si["on_wait"] = [waits[-1]]
                out_insts.append(inst)
            if changed:
                blk["instructions"] = out_insts
    return json.dumps(bir).encode()


def build_nc(debug: bool = False) -> Bass:
    _patch_tile_drain()
    nc = Bass()
    AX = mybir.AxisListType
    OP = mybir.AluOpType

    x = nc.dram_tensor("x", [NQ, D_FEAT], F32, kind="ExternalInput")
    ktrh = nc.dram_tensor("ktrh", [128, NBLK3 * MMN], BF16, kind="ExternalInput")
    xaugT = nc.dram_tensor("xaugT", [128, NQ], F32, kind="ExternalInput")
    mvpad = nc.dram_tensor("mvpad", [N_MEM, 16], F32, kind="ExternalInput")
    w = nc.dram_tensor("w", [1, D_FEAT], F32, kind="ExternalInput")
    b = nc.dram_tensor("b", [1], F32, kind="ExternalInput")
    out = nc.dram_tensor("out", [NQ, 1], F32, kind="ExternalOutput")

    with tile.TileContext(nc) as tc:
        with (
            tc.tile_pool(name="sbuf", bufs=1) as pool,
            tc.tile_pool(name="work", bufs=4) as wpool,
            tc.tile_pool(name="psum", bufs=2, space="PSUM") as ppool,
        ):
            # ---- loads ------------------------------------------------------
            # Keys spread over all 128 SBUF partitions as four 16-row aug
            # blocks at partition bases {0,32,64,96}: full-bandwidth DMA, and
            # the 4 matmuls of a tile rotate PE row groups so LDWEIGHTS of
            # MM i+1 overlaps MM i (no row-group conflict).
            # q = t*4 + k (512-col block index): partition base 32*(q%3),
            # col slot (q//3)*512. PE quadrant 3 (base 96) is unusable.
            ktr_t = pool.tile([128, NBLK3 * MMN], BF16, tag="ktr")
            nc.sync.dma_start(out=ktr_t[:], in_=ktrh[:])

            xaT_f = pool.tile([128, NQ], F32, tag="xaTf")
            nc.sync.dma_start(out=xaT_f[:], in_=xaugT[:])
            xaT = pool.tile([128, NQ], BF16, tag="xaT")
            nc.vector.tensor_copy(out=xaT[:], in_=xaT_f[:])

            # x in layout B (q = g*128 + m): for the linear fallback
            xqb_t = pool.tile([128, QS * D_FEAT], F32, tag="xqb")
            nc.sync.dma_start(
                out=xqb_t[:].rearrange("p (g d) -> p g d", d=D_FEAT),
                in_=x[:].rearrange("(g m) d -> m g d", m=128),
            )
            xqb_v = xqb_t[:].rearrange("p (g d) -> p g d", d=D_FEAT)

            w_t = pool.tile([128, D_FEAT], F32, tag="wt")
            nc.sync.dma_start(out=w_t[:], in_=w[0:1, :].to_broadcast([128, D_FEAT]))
            b_t = pool.tile([128, 1], F32, tag="bt")
            nc.sync.dma_start(out=b_t[:], in_=b[None, :].to_broadcast([128, 1]))

            # ---- linear fallback linq[m, g] = x_q . w + b -------------------
            xw_t = pool.tile([128, QS * D_FEAT], F32, tag="xw")
            nc.vector.tensor_tensor(
                out=xw_t[:].rearrange("p (g d) -> p g d", d=D_FEAT),
                in0=xqb_v,
                in1=w_t[:, None, :].to_broadcast([128, QS, D_FEAT]),
                op=OP.mult,
            )
            linq_t = pool.tile([128, QS], F32, tag="linq")
            nc.vector.reduce_sum(
                out=linq_t[:],
                in_=xw_t[:].rearrange("p (g d) -> p g d", d=D_FEAT),
                axis=AX.X,
            )
            nc.vector.tensor_scalar_add(linq_t[:], linq_t[:], b_t[:, 0:1])

            # ---- main loop: matmul -> fused fold+min ------------------------
            # mins[m, g*16 + t] = min over tile t of group g
            mins_t = pool.tile([128, QS * NTILE], F32, tag="mins")

            val_t = pool.tile([128, QS], F32, tag="val")
            kk_t = pool.tile([128, QS], F32, tag="kk")
            ti_t = pool.tile([128, NTILE], I32, tag="ti")
            nc.gpsimd.iota(ti_t[:], [[1, NTILE]], base=0, channel_multiplier=0)
            tbase_t = pool.tile([128, NTILE], F32, tag="tbase")
            nc.vector.tensor_copy(out=tbase_t[:], in_=ti_t[:])
            nc.vector.tensor_scalar(
                out=tbase_t[:], in0=tbase_t[:], scalar1=float(TW), scalar2=1.0,
                op0=OP.mult, op1=OP.add,
            )  # t*2048 + 1

            for g in range(QS):
                for t in range(NTILE):
                    ps = ppool.tile([128, TW], F32, tag="ps")
                    for k in range(TW // MMN):
                        q = t * 4 + k
                        pb = 32 * (q % 3)
                        sl = q // 3
                        nc.tensor.matmul(
                            out=ps[:, k * MMN : (k + 1) * MMN],
                            lhsT=xaT[pb : pb + KAUG, g * 128 : (g + 1) * 128],
                            rhs=ktr_t[
                                pb : pb + KAUG, sl * MMN : (sl + 1) * MMN
                            ],
                            start=True,
                            stop=True,
                        )
                    nc.vector.tensor_reduce(
                        out=mins_t[:, g * NTILE + t : g * NTILE + t + 1],
                        in_=ps[:, None, :],
                        axis=AX.X,
                        op=OP.min,
                    )

                # ---- per-group decode + gather ------------------------------
                gm = mins_t[:, g * NTILE : (g + 1) * NTILE]
                fo = wpool.tile([128, NTILE], F32, tag="fo")
                nc.vector.tensor_scalar(
                    out=fo[:], in0=gm, scalar1=0.5, scalar2=None, op0=OP.is_lt
                )
                ii = wpool.tile([128, NTILE], F32, tag="ii")
                nc.vector.tensor_scalar(
                    out=ii[:], in0=gm, scalar1=-8192.0, scalar2=float(TW),
                    op0=OP.mult, op1=OP.add,
                )  # i* = 2048 - m*8192
                nc.vector.tensor_tensor(out=ii[:], in0=ii[:], in1=tbase_t[:], op=OP.add)
                nc.vector.tensor_tensor(out=ii[:], in0=ii[:], in1=fo[:], op=OP.mult)
                nc.vector.reduce_max(
                    out=kk_t[:, g : g + 1], in_=ii[:], axis=AX.X
                )  # K = jstar + 1, 0 if not found

                rg = pool.tile([128, 1], F32, name=f"rgf{g}", tag=f"rgf{g}")
                nc.vector.tensor_scalar(
                    out=rg[:], in0=kk_t[:, g : g + 1], scalar1=-1.0, scalar2=0.0,
                    op0=OP.add, op1=OP.max,
                )
                rgi = pool.tile([128, 1], I32, name=f"rgi{g}", tag=f"rgi{g}")
                nc.vector.tensor_copy(out=rgi[:], in_=rg[:])
                vg = pool.tile([128, 16], F32, name=f"vg{g}", tag=f"vg{g}")
                nc.gpsimd.indirect_dma_start(
                    out=vg[:],
                    out_offset=None,
                    in_=mvpad[:],
                    in_offset=IndirectOffsetOnAxis(ap=rgi[:, 0:1], axis=0),
                )
                nc.vector.tensor_copy(out=val_t[:, g : g + 1], in_=vg[:, 0:1])

                # select + store this group (overlaps next group's compute)
                found_g = pool.tile([128, 1], F32, name=f"fnd{g}", tag=f"fnd{g}")
                nc.vector.tensor_scalar(
                    out=found_g[:], in0=kk_t[:, g : g + 1], scalar1=0.0,
                    scalar2=None, op0=OP.is_gt,
                )
                found_u8 = pool.tile([128, 1], U8, name=f"fu8{g}", tag=f"fu8{g}")
                nc.vector.tensor_copy(out=found_u8[:], in_=found_g[:])
                res_g = pool.tile([128, 1], F32, name=f"res{g}", tag=f"res{g}")
                nc.vector.select(
                    out=res_g[:],
                    mask=found_u8[:],
                    on_true=val_t[:, g : g + 1],
                    on_false=linq_t[:, g : g + 1],
                )
                nc.sync.dma_start(
                    out=out[g * 128 : (g + 1) * 128, :], in_=res_g[:]
                )

            if debug:
                taps = {
                    "d_mins": mins_t,
                    "d_kk": kk_t,
                    "d_linq": linq_t,
                    "d_val": val_t,
                }
                for name, t_ in taps.items():
                    shp = list(t_[:].shape)
                    dt_ = nc.dram_tensor(name, shp, F32, kind="ExternalOutput")
                    nc.sync.dma_start(out=dt_[:], in_=t_[:])

    return nc


_NC_CACHE: dict[str, Bass] = {}


def _get_nc() -> Bass:
    if "nc" not in _NC_CACHE:
        nc = build_nc()
        orig = nc.to_json_bytes
        nc.to_json_bytes = lambda: _fix_multiwaits(orig())
        _NC_CACHE["nc"] = nc
    return _NC_CACHE["nc"]


def _prep_keys(mem_keys: np.ndarray, mem_values: np.ndarray):
    """Host-side layout prep: contraction-major augmented key matrix (bf16,
    all values exactly representable) and 64B-padded value rows."""
    import ml_dtypes

    j = np.arange(N_MEM)
    i_loc = j % TW
    A = (i_loc >> 5).astype(np.float32)
    B = (i_loc & 31).astype(np.float32)
    ktr = np.zeros((KAUG, N_MEM), dtype=np.float32)
    ktr[0:D_FEAT, :] = mem_keys.T
    ktr[8, :] = (mem_keys * mem_keys).sum(axis=1)
    ktr[9, :] = 0.25
    ktr[10, :] = -A * 2.0 ** -8
    ktr[11, :] = -B * 2.0 ** -13
    ktr[12, :] = 1.0
    # spread over partitions: 512-col block q = j//512 goes to partition
    # base 32*(q%3), col slot (q//3)*512 (3-way PE row-group rotation)
    ktrh = np.zeros((128, NBLK3 * 512), dtype=np.float32)
    q_i, i_i = j // 512, j % 512
    for r in range(KAUG):
        ktrh[32 * (q_i % 3) + r, (q_i // 3) * 512 + i_i] = ktr[r, j]
    ktrh = ktrh.astype(ml_dtypes.bfloat16)
    mvpad = np.ascontiguousarray(
        np.repeat(mem_values[:, None], 16, axis=1).astype(np.float32)
    )
    return ktrh, mvpad


def _prep_queries(xc: np.ndarray):
    """Host-side query-side augmented columns, replicated at the four
    partition bases to match the key row groups: [128, 512] f32."""
    xa = np.zeros((KAUG, NQ), dtype=np.float32)
    xa[0:D_FEAT, :] = -2.0 * xc.T
    xa[8, :] = 1.0
    xa[9, :] = 1.0
    xa[10, :] = 1.0
    xa[11, :] = 1.0
    xa[12, :] = (xc * xc).sum(axis=1)
    xa4 = np.zeros((128, NQ), dtype=np.float32)
    for k in range(3):
        xa4[32 * k : 32 * k + KAUG, :] = xa
    return xa4


def kernel(x, mem_keys, mem_values, w, b):
    from concourse.bass_utils import run_bass_kernel_spmd

    x = np.ascontiguousarray(np.asarray(x, dtype=np.float32))
    mem_keys = np.ascontiguousarray(np.asarray(mem_keys, dtype=np.float32))
    mem_values = np.ascontiguousarray(np.asarray(mem_values, dtype=np.float32))
    w = np.ascontiguousarray(np.asarray(w, dtype=np.float32))
    b = np.ascontiguousarray(np.asarray(b, dtype=np.float32))

    nc = _get_nc()
    ktrh, mvpad = _prep_keys(mem_keys, mem_values)
    in_maps = []
    for c in range(N_CORES):
        xc = x[c * NQ : (c + 1) * NQ]
        in_maps.append(
            {
                "x": xc,
                "ktrh": ktrh,
                "xaugT": _prep_queries(xc),
                "mvpad": mvpad,
                "w": w,
                "b": b,
            }
        )
    res = run_bass_kernel_spmd(nc, in_maps, core_ids=list(range(N_CORES)))
    return np.concatenate([r["out"] for r in res.results], axis=0)


if __name__ == "__main__":
    rng = np.random.default_rng(0)
    mk = rng.integers(0, 4, (N_MEM, D_FEAT)).astype(np.float32)
    xx = rng.integers(0, 4, (N_QUERIES, D_FEAT)).astype(np.float32)
    mv = rng.normal(size=N_MEM).astype(np.float32)
    ww = rng.normal(size=(1, D_FEAT)).astype(np.float32)
    bb = rng.normal(size=(1,)).astype(np.float32)
    got = kernel(xx, mk, mv, ww, bb)
    pow4 = (4 ** np.arange(D_FEAT)).astype(np.int64)
    mc = (mk.astype(np.int64) * pow4).sum(1)
    qc = (xx.astype(np.int64) * pow4).sum(1)
    last = {}
    for jj, c in enumerate(mc):
        last[c] = jj
    exp = np.where(
        np.isin(qc, mc),
        mv[[last.get(c, 0) for c in qc]],
        (xx @ ww.T + bb)[:, 0],
    )[:, None]
    err = np.abs(got - exp).max()
    print("max abs err vs numpy model:", err)
